# Initial kernel scaffold
#
"""Trainium2 Bass kernel for nn_DecoderRNN (highway-LSTM decoder).

Strategy (8 NeuronCores, tensor-parallel over the hidden dimension):
  - H=1024 is split into 8 chunks of 128; core c owns hidden units
    [c*128, (c+1)*128) for the gate math and the recurrent state.
  - Input projection pi = x @ W_in.T is column-sharded the same way:
    core c computes the 6 gate chunks (6*128=768 rows of W_in) it needs,
    for the full batch, for all timesteps -- fused into the step loop.
  - Recurrent projection ps = h @ W_state.T (5*128=640 rows per core)
    accumulates into the same PSUM banks as pi, so the gate
    pre-activations come out of PSUM ready for the activations.
  - After each step the 8 cores all-gather their h.T chunks [128, B]
    so every core has the full h.T [1024, B] for the next step.
  - The output projection logits = h @ W_out.T is K-sharded (each core
    contracts its 128 hidden units); partial logits are summed on the
    host -- no final collective.

All matmuls run as float32r (TF32-like, full PE rate at N>=256).
Layouts keep the batch (256) as the matmul free dimension and hidden/
gate indices on partitions, so h.T comes out of the gate math in
exactly the layout the next step's matmul consumes -- no transposes
anywhere on the device. x is pre-transposed on the host.
"""
import numpy as np

import concourse.bass as bass
import concourse.bacc as bacc
import concourse.mybir as mybir
import concourse.tile as tile
from concourse.bass_utils import run_bass_kernel_spmd

F32 = mybir.dt.float32
F32R = mybir.dt.float32r
AF = mybir.ActivationFunctionType
OP = mybir.AluOpType

T, B, DIN, H, C = 32, 256, 4196, 1024, 151
NCORES = 8
HC = H // NCORES            # 128 hidden units per core
KX = 33                     # ceil(4196 / 128) input k-chunks
DINP = KX * 128             # 4224 padded input dim
CP = 256                    # padded class dim

_CACHE = {}


def build_nc(n_steps=T):
    nc = bacc.Bacc("TRN2", target_bir_lowering=False, debug=False,
                   num_devices=NCORES)

    xT = nc.dram_tensor("xT", [n_steps, KX, 128, B], F32, kind="ExternalInput")
    w_in = nc.dram_tensor("w_in", [KX, 128, 6 * HC], F32, kind="ExternalInput")
    w_st = nc.dram_tensor("w_st", [NCORES, 128, 5 * HC], F32, kind="ExternalInput")
    w_out = nc.dram_tensor("w_out", [128, CP], F32, kind="ExternalInput")
    bias = nc.dram_tensor("bias", [128, 8], F32, kind="ExternalInput")
    maskT = nc.dram_tensor("maskT", [128, B], F32, kind="ExternalInput")
    c0T = nc.dram_tensor("c0T", [128, B], F32, kind="ExternalInput")
    h0T = nc.dram_tensor("h0T", [NCORES, 128, B], F32, kind="ExternalInput")
    out = nc.dram_tensor("out", [n_steps, 2, 128, CP], F32, kind="ExternalOutput")

    rg = [list(range(NCORES))]

    with tile.TileContext(nc) as tc:
        with (
            tc.tile_pool(name="const", bufs=1) as cpool,
            tc.tile_pool(name="xp", bufs=20) as xp,
            tc.tile_pool(name="ht", bufs=16) as htp,
            tc.tile_pool(name="gt", bufs=1) as gt,
            tc.tile_pool(name="st", bufs=2) as st,
            tc.tile_pool(name="lgs", bufs=4) as lgs,
            tc.tile_pool(name="pi", bufs=6, space="PSUM") as pip,
            tc.tile_pool(name="lg", bufs=2, space="PSUM") as lgp,
            tc.tile_pool(name="dram", bufs=4, space="DRAM") as dram,
        ):
            # ---- resident weights / constants ----
            w_in_sb = cpool.tile([128, KX * 6 * HC], F32R)
            for k in range(KX):
                nc.sync.dma_start(
                    out=w_in_sb[:, k * 768:(k + 1) * 768],
                    in_=w_in[k].bitcast(F32R))
            w_st_sb = cpool.tile([128, NCORES * 5 * HC], F32R)
            for k in range(NCORES):
                nc.sync.dma_start(
                    out=w_st_sb[:, k * 640:(k + 1) * 640],
                    in_=w_st[k].bitcast(F32R))
            w_out_sb = cpool.tile([128, CP], F32R)
            nc.sync.dma_start(out=w_out_sb[:], in_=w_out[:].bitcast(F32R))
            bias_sb = cpool.tile([128, 8], F32)
            nc.sync.dma_start(out=bias_sb[:], in_=bias[:])
            mask_sb = cpool.tile([128, B], F32)
            nc.sync.dma_start(out=mask_sb[:], in_=maskT[:])

            # ---- state ----
            c_prev = st.tile([128, B], F32, tag="c")
            nc.sync.dma_start(out=c_prev[:], in_=c0T[:])
            ht_tiles = []
            for k in range(NCORES):
                t_ = htp.tile([128, B], F32R, tag="ht")
                nc.sync.dma_start(out=t_[:], in_=h0T[k].bitcast(F32R))
                ht_tiles.append(t_)

            def emit_pi(t):
                """198 MMs: pi for step t into 3 fresh PSUM banks."""
                banks = [pip.tile([128, 512], F32, tag="pi") for _ in range(3)]
                for k in range(KX):
                    xt = xp.tile([128, B], F32R, tag="x")
                    nc.sync.dma_start(out=xt[:], in_=xT[t, k].bitcast(F32R))
                    for g in range(6):
                        j, half = g // 2, g % 2
                        nc.tensor.matmul(
                            banks[j][:, half * B:(half + 1) * B],
                            w_in_sb[:, k * 768 + g * 128: k * 768 + (g + 1) * 128],
                            xt[:],
                            start=(k == 0),
                            stop=(k == KX - 1 and g == 5),
                            skip_group_check=True,
                        )
                return banks

            def emit_ps(banks, ht_in):
                """40 MMs: ps for this step accumulated onto the pi banks."""
                for k in range(NCORES):
                    for g in range(5):
                        j, half = g // 2, g % 2
                        nc.tensor.matmul(
                            banks[j][:, half * B:(half + 1) * B],
                            w_st_sb[:, k * 640 + g * 128: k * 640 + (g + 1) * 128],
                            ht_in[k][:],
                            start=False,
                            stop=(k == NCORES - 1),
                            skip_group_check=True,
                        )

            def bsl(g):
                return bias_sb[:, g:g + 1]

            pi_banks = emit_pi(0)
            pending_logits = []  # (t, h_r tile) awaiting output projection

            def emit_logits(t, h_r):
                for half in range(2):
                    lg_ps = lgp.tile([128, CP], F32, tag="lg")
                    nc.tensor.matmul(
                        lg_ps[:],
                        h_r[:, half * 128:(half + 1) * 128],
                        w_out_sb[:],
                        start=True, stop=True,
                    )
                    lg_sb = lgs.tile([128, CP], F32, tag="lgs")
                    nc.vector.tensor_copy(lg_sb[:], lg_ps[:])
                    nc.sync.dma_start(out=out[t, half], in_=lg_sb[:])

            for t in range(n_steps):
                # pi for the next step first: keeps PE busy while this
                # step's all-gather is in flight.
                next_banks = emit_pi(t + 1) if t + 1 < n_steps else None
                if pending_logits:
                    emit_logits(*pending_logits.pop())

                emit_ps(pi_banks, ht_tiles)

                # gate activations straight out of PSUM
                pb0, pb1, pb2 = pi_banks
                i_g = gt.tile([128, B], F32, tag="i")
                f_g = gt.tile([128, B], F32, tag="f")
                m_i = gt.tile([128, B], F32, tag="m")
                o_g = gt.tile([128, B], F32, tag="o")
                hw = gt.tile([128, B], F32, tag="hw")
                pi5 = gt.tile([128, B], F32, tag="pi5")
                nc.scalar.activation(i_g[:], pb0[:, 0:B], AF.Sigmoid, bias=bsl(0))
                nc.scalar.activation(f_g[:], pb0[:, B:2 * B], AF.Sigmoid, bias=bsl(1))
                nc.scalar.activation(m_i[:], pb1[:, 0:B], AF.Tanh, bias=bsl(2))
                nc.scalar.activation(o_g[:], pb1[:, B:2 * B], AF.Sigmoid, bias=bsl(3))
                nc.scalar.activation(hw[:], pb2[:, 0:B], AF.Sigmoid, bias=bsl(4))
                nc.scalar.activation(pi5[:], pb2[:, B:2 * B], AF.Identity, bias=bsl(5))

                t1 = gt.tile([128, B], F32, tag="t1")
                nc.vector.tensor_mul(t1[:], i_g[:], m_i[:])
                t2 = gt.tile([128, B], F32, tag="t2")
                nc.vector.tensor_mul(t2[:], f_g[:], c_prev[:])
                c_new = st.tile([128, B], F32, tag="c")
                nc.vector.tensor_add(c_new[:], t1[:], t2[:])
                tm = gt.tile([128, B], F32, tag="tm")
                nc.scalar.activation(tm[:], c_new[:], AF.Tanh)
                t3 = gt.tile([128, B], F32, tag="t3")
                nc.vector.tensor_mul(t3[:], o_g[:], tm[:])
                t4 = gt.tile([128, B], F32, tag="t4")
                nc.vector.tensor_sub(t4[:], t3[:], pi5[:])
                t5 = gt.tile([128, B], F32, tag="t5")
                nc.vector.tensor_mul(t5[:], hw[:], t4[:])
                t6 = gt.tile([128, B], F32, tag="t6")
                nc.vector.tensor_add(t6[:], t5[:], pi5[:])
                h_r = st.tile([128, B], F32R, tag="hr")
                nc.vector.tensor_mul(h_r[:], t6[:], mask_sb[:].bitcast(F32R))
                c_prev = c_new

                pending_logits.append((t, h_r))

                if t + 1 < n_steps:
                    # all-gather h.T chunks for the next step
                    bin_ = dram.tile([128, B], F32, tag="bin")
                    nc.sync.dma_start(out=bin_[:], in_=h_r[:].bitcast(F32))
                    bout = dram.tile([NCORES * 128, B], F32, tag="bout")
                    nc.gpsimd.collective_compute(
                        "AllGather", OP.bypass,
                        replica_groups=rg,
                        ins=[bin_.opt()], outs=[bout.opt()],
                    )
                    ht_tiles = []
                    for k in range(NCORES):
                        t_ = htp.tile([128, B], F32R, tag="ht")
                        nc.sync.dma_start(
                            out=t_[:],
                            in_=bout[k * 128:(k + 1) * 128, :].bitcast(F32R))
                        ht_tiles.append(t_)
                    pi_banks = next_banks

            emit_logits(*pending_logits.pop())

    nc.compile()
    return nc


def _prep_inputs(x, h0, c0, dropout_mask, W_in, b_in, W_state, b_state,
                 W_out, b_out):
    """Host-side shard + transpose + pad. Returns per-core input maps."""
    n_steps = x.shape[0]
    # x [T,B,DIN] -> [T, KX, 128, B]
    xp = np.zeros((n_steps, DINP, B), dtype=np.float32)
    xp[:, :DIN, :] = np.ascontiguousarray(x.transpose(0, 2, 1))
    xT = np.ascontiguousarray(xp.reshape(n_steps, KX, 128, B))

    h0T = np.ascontiguousarray(h0.T.reshape(NCORES, 128, B)).astype(np.float32)

    in_maps = []
    for c in range(NCORES):
        sl = slice(c * HC, (c + 1) * HC)
        # W_in rows for this core's 6 gate chunks -> [KX,128, 6*HC]
        wi = np.concatenate([W_in[g * H + c * HC:g * H + (c + 1) * HC]
                             for g in range(6)], axis=0)  # [768, DIN]
        wip = np.zeros((768, DINP), dtype=np.float32)
        wip[:, :DIN] = wi
        w_in_c = np.ascontiguousarray(wip.T.reshape(KX, 128, 768))

        ws = np.concatenate([W_state[g * H + c * HC:g * H + (c + 1) * HC]
                             for g in range(5)], axis=0)  # [640, H]
        w_st_c = np.ascontiguousarray(ws.T.reshape(NCORES, 128, 640))

        wo = np.zeros((128, CP), dtype=np.float32)
        wo[:, :C] = W_out[:, sl].T
        bias_c = np.zeros((128, 8), dtype=np.float32)
        for g in range(6):
            bias_c[:, g] = b_in[g * H + c * HC:g * H + (c + 1) * HC]
            if g < 5:
                bias_c[:, g] += b_state[g * H + c * HC:g * H + (c + 1) * HC]

        in_maps.append({
            "xT": xT,
            "w_in": w_in_c,
            "w_st": w_st_c,
            "w_out": wo,
            "bias": bias_c,
            "maskT": np.ascontiguousarray(dropout_mask.T[sl]).astype(np.float32),
            "c0T": np.ascontiguousarray(c0.T[sl]).astype(np.float32),
            "h0T": h0T,
        })
    return in_maps


def kernel(x, h0, c0, dropout_mask, W_in, b_in, W_state, b_state,
           W_out, b_out, _trace=False):
    n_steps = x.shape[0]
    if n_steps not in _CACHE:
        _CACHE[n_steps] = build_nc(n_steps)
    nc = _CACHE[n_steps]
    in_maps = _prep_inputs(np.asarray(x, dtype=np.float32),
                           np.asarray(h0, dtype=np.float32),
                           np.asarray(c0, dtype=np.float32),
                           np.asarray(dropout_mask, dtype=np.float32),
                           np.asarray(W_in, dtype=np.float32),
                           np.asarray(b_in, dtype=np.float32),
                           np.asarray(W_state, dtype=np.float32),
                           np.asarray(b_state, dtype=np.float32),
                           np.asarray(W_out, dtype=np.float32),
                           np.asarray(b_out, dtype=np.float32))
    res = run_bass_kernel_spmd(nc, in_maps, list(range(NCORES)), trace=_trace)
    acc = np.zeros((n_steps, 2, 128, CP), dtype=np.float64)
    for r in res.results:
        acc += r["out"]
    logits = acc.reshape(n_steps, B, CP)[:, :, :C].astype(np.float32)
    logits += np.asarray(b_out, dtype=np.float32)
    kernel.last_result = res
    return logits


# revision 12
# speedup vs baseline: 1.1652x; 1.1652x over previous
"""Trainium2 Bass kernel for nn_DecoderRNN (highway-LSTM decoder).

Strategy (8 NeuronCores, tensor-parallel over the hidden dimension):
  - H=1024 is split into 8 chunks of 128; core c owns hidden units
    [c*128, (c+1)*128) for the gate math and the recurrent state.
  - Input projection pi = x @ W_in.T is column-sharded the same way:
    core c computes the 6 gate chunks (6*128=768 rows of W_in) it
    needs, for the full batch, fused into the step loop. It runs in
    bf16 at N=512 (two timesteps per moving block) so the PE streams
    at full rate with fast weight load; results accumulate in fp32
    PSUM and are copied to SBUF right away to free the banks.
  - Recurrent projection ps = h @ W_state.T (5*128=640 rows per core)
    runs in float32r (TF32-like) for recurrence accuracy, into two
    rotating work banks; gate pre-activations = pi(SBUF) + ps(PSUM).
  - After each step the 8 cores all-gather their h.T chunks [128, B]
    so every core has the full h.T [1024, B] for the next step.
  - The output projection logits = h @ W_out.T is K-sharded (each core
    contracts its own 128 hidden units); partial logits are summed on
    the host -- no final collective.

Layouts keep batch (256) as the matmul free dimension and hidden/gate
indices on partitions, so h.T comes out of the gate math in exactly
the layout the next step's matmul consumes -- no device transposes.
x is pre-transposed and bf16-cast on the host.
"""
import numpy as np

import concourse.bass as bass
import concourse.bacc as bacc
import concourse.mybir as mybir
import concourse.tile as tile
from concourse.bass_utils import run_bass_kernel_spmd

F32 = mybir.dt.float32
F32R = mybir.dt.float32r
BF16 = mybir.dt.bfloat16
AF = mybir.ActivationFunctionType
OP = mybir.AluOpType

T, B, DIN, H, C = 32, 256, 4196, 1024, 151
NCORES = 8
HC = H // NCORES            # 128 hidden units per core
KX = 33                     # ceil(4196 / 128) input k-chunks
DINP = KX * 128             # 4224 padded input dim
CP = 256                    # padded class dim

_CACHE = {}
DEBUG = False


def build_nc(n_steps=T):
    assert n_steps % 2 == 0
    n_grp = n_steps // 2
    nc = bacc.Bacc("TRN2", target_bir_lowering=False, debug=False,
                   num_devices=NCORES)

    # x blocks: group g holds steps (2g, 2g+1) side by side in the free dim
    xT = nc.dram_tensor("xT", [n_grp, KX, 128, 2 * B], BF16, kind="ExternalInput")
    w_in = nc.dram_tensor("w_in", [KX, 128, 6 * HC], BF16, kind="ExternalInput")
    w_st = nc.dram_tensor("w_st", [NCORES, 128, 5 * HC], BF16, kind="ExternalInput")
    w_out = nc.dram_tensor("w_out", [128, CP], F32, kind="ExternalInput")
    bias = nc.dram_tensor("bias", [128, 8], F32, kind="ExternalInput")
    maskT = nc.dram_tensor("maskT", [128, B], F32, kind="ExternalInput")
    c0T = nc.dram_tensor("c0T", [128, B], F32, kind="ExternalInput")
    h0T = nc.dram_tensor("h0T", [NCORES, 128, B], BF16, kind="ExternalInput")
    out = nc.dram_tensor("out", [n_steps, 2, 128, CP], F32, kind="ExternalOutput")
    if DEBUG:
        dbg_h = nc.dram_tensor("dbg_h", [n_steps, 128, B], F32,
                               kind="ExternalOutput")

    rg = [list(range(NCORES))]

    with tile.TileContext(nc) as tc:
        with (
            tc.tile_pool(name="const", bufs=1) as cpool,
            tc.tile_pool(name="wi", bufs=KX) as wip,
            tc.tile_pool(name="xp", bufs=24) as xp,
            tc.tile_pool(name="pisb", bufs=2) as pisb,
            tc.tile_pool(name="ht", bufs=16) as htp,
            tc.tile_pool(name="gt", bufs=1) as gt,
            tc.tile_pool(name="st", bufs=2) as st,
            tc.tile_pool(name="lgs", bufs=4) as lgs,
            tc.tile_pool(name="pi", bufs=1, space="PSUM") as pip,
            tc.tile_pool(name="wk", bufs=2, space="PSUM") as wkp,
            tc.tile_pool(name="dram", bufs=4, space="DRAM") as dram,
        ):
            # ---- resident weights / constants ----
            wi_tiles = []
            for k in range(KX):
                w_ = wip.tile([128, 6 * HC], BF16, tag="wi", name=f"wi{k}")
                nc.gpsimd.dma_start(out=w_[:], in_=w_in[k])
                wi_tiles.append(w_)
            w_st_sb = cpool.tile([128, NCORES * 5 * HC], BF16)
            for k in range(NCORES):
                nc.sync.dma_start(
                    out=w_st_sb[:, k * 640:(k + 1) * 640],
                    in_=w_st[k])
            w_out_sb = cpool.tile([128, CP], F32R)
            nc.sync.dma_start(out=w_out_sb[:], in_=w_out[:].bitcast(F32R))
            bias_sb = cpool.tile([128, 8], F32)
            nc.sync.dma_start(out=bias_sb[:], in_=bias[:])
            mask_sb = cpool.tile([128, B], F32)
            nc.sync.dma_start(out=mask_sb[:], in_=maskT[:])

            # ---- state ----
            c_prev = st.tile([128, B], F32, tag="c")
            nc.sync.dma_start(out=c_prev[:], in_=c0T[:])
            ht_tiles = []
            for k in range(NCORES):
                t_ = htp.tile([128, B], BF16, tag="ht", name=f"ht0_{k}")
                nc.sync.dma_start(out=t_[:], in_=h0T[k])
                ht_tiles.append(t_)

            def emit_pi_half(g, lo, hi):
                """bf16 N=512 matmuls for k-chunks [lo,hi) of group g."""
                banks = pi_banks_cur[g % 2]
                for k in range(lo, hi):
                    xt = xp.tile([128, 2 * B], BF16, tag="x", name=f"x{g}_{k}")
                    nc.gpsimd.dma_start(out=xt[:], in_=xT[g, k])
                    for gg in range(6):
                        nc.tensor.matmul(
                            banks[gg][:],
                            wi_tiles[k][:, gg * 128:(gg + 1) * 128],
                            xt[:],
                            start=(k == 0),
                            stop=(k == KX - 1),
                            skip_group_check=True,
                        )

            def copy_pi_out(g):
                banks = pi_banks_cur[g % 2]
                tiles = []
                for gg in range(6):
                    p_ = pisb.tile([128, 2 * B], F32, tag=f"pisb{gg}",
                                   name=f"pisb{g}_{gg}")
                    nc.vector.tensor_copy(p_[:], banks[gg][:])
                    tiles.append(p_)
                return tiles

            def bsl(g):
                return bias_sb[:, g:g + 1]

            def emit_step(t, pi_tiles, half):
                """recurrence for step t; pre-act = pi_tiles[g][:,half] + ps."""
                nonlocal c_prev, ht_tiles
                hof = half * B
                # ps waves into the 2 work banks: gates (0,1) and (2,3),
                # then gate 4 reuses the first slot.
                wk1 = wkp.tile([128, 512], F32, tag="wk", name=f"wkA{t}")
                wk2 = wkp.tile([128, 512], F32, tag="wk", name=f"wkB{t}")
                for k in range(NCORES):
                    for gg in range(4):
                        wk_ = wk1 if gg < 2 else wk2
                        co = (gg % 2) * B
                        nc.tensor.matmul(
                            wk_[:, co:co + B],
                            w_st_sb[:, k * 640 + gg * 128: k * 640 + (gg + 1) * 128],
                            ht_tiles[k][:],
                            start=(k == 0 and gg % 2 == 0),
                            stop=(k == NCORES - 1),
                            skip_group_check=True,
                        )
                pre = {}
                for gg, nm in ((0, "i"), (1, "f"), (2, "m"), (3, "o")):
                    wk_ = wk1 if gg < 2 else wk2
                    co = (gg % 2) * B
                    p_ = gt.tile([128, B], F32, tag=f"pre{nm}", name=f"pre{nm}{t}")
                    nc.vector.tensor_add(p_[:], wk_[:, co:co + B],
                                         pi_tiles[gg][:, hof:hof + B])
                    pre[nm] = p_
                # wave C: gate 4 (hw) into a fresh slot (reuses wk1's bank)
                wk3 = wkp.tile([128, 512], F32, tag="wk", name=f"wkC{t}")
                for k in range(NCORES):
                    nc.tensor.matmul(
                        wk3[:, 0:B],
                        w_st_sb[:, k * 640 + 4 * 128: k * 640 + 5 * 128],
                        ht_tiles[k][:],
                        start=(k == 0),
                        stop=(k == NCORES - 1),
                        skip_group_check=True,
                    )
                phw = gt.tile([128, B], F32, tag="prehw", name=f"prehw{t}")
                nc.vector.tensor_add(phw[:], wk3[:, 0:B],
                                     pi_tiles[4][:, hof:hof + B])

                i_g = gt.tile([128, B], F32, tag="i", name=f"i{t}")
                f_g = gt.tile([128, B], F32, tag="f", name=f"f{t}")
                m_i = gt.tile([128, B], F32, tag="m", name=f"mm{t}")
                o_g = gt.tile([128, B], F32, tag="o", name=f"o{t}")
                hw = gt.tile([128, B], F32, tag="hw", name=f"hw{t}")
                pi5 = gt.tile([128, B], F32, tag="pi5", name=f"pi5{t}")
                nc.scalar.activation(i_g[:], pre["i"][:], AF.Sigmoid, bias=bsl(0))
                nc.scalar.activation(f_g[:], pre["f"][:], AF.Sigmoid, bias=bsl(1))
                nc.scalar.activation(m_i[:], pre["m"][:], AF.Tanh, bias=bsl(2))
                nc.scalar.activation(o_g[:], pre["o"][:], AF.Sigmoid, bias=bsl(3))
                nc.scalar.activation(hw[:], phw[:], AF.Sigmoid, bias=bsl(4))
                nc.scalar.activation(pi5[:], pi_tiles[5][:, hof:hof + B],
                                     AF.Identity, bias=bsl(5))

                t1 = gt.tile([128, B], F32, tag="t1", name=f"t1{t}")
                nc.vector.tensor_mul(t1[:], i_g[:], m_i[:])
                t2 = gt.tile([128, B], F32, tag="t2", name=f"t2{t}")
                nc.vector.tensor_mul(t2[:], f_g[:], c_prev[:])
                c_new = st.tile([128, B], F32, tag="c", name=f"c{t}")
                nc.vector.tensor_add(c_new[:], t1[:], t2[:])
                tm = gt.tile([128, B], F32, tag="tm", name=f"tm{t}")
                nc.scalar.activation(tm[:], c_new[:], AF.Tanh)
                t3 = gt.tile([128, B], F32, tag="t3", name=f"t3{t}")
                nc.vector.tensor_mul(t3[:], o_g[:], tm[:])
                t4 = gt.tile([128, B], F32, tag="t4", name=f"t4{t}")
                nc.vector.tensor_sub(t4[:], t3[:], pi5[:])
                t5 = gt.tile([128, B], F32, tag="t5", name=f"t5{t}")
                nc.vector.tensor_mul(t5[:], hw[:], t4[:])
                t6 = gt.tile([128, B], F32, tag="t6", name=f"t6{t}")
                nc.vector.tensor_add(t6[:], t5[:], pi5[:])
                h_f = gt.tile([128, B], F32, tag="hf", name=f"hf{t}")
                nc.vector.tensor_mul(h_f[:], t6[:], mask_sb[:])
                h_r = st.tile([128, B], BF16, tag="hr", name=f"hr{t}")
                nc.vector.tensor_copy(h_r[:], h_f[:])
                h_r32 = st.tile([128, B], F32R, tag="hr32", name=f"hr32_{t}")
                nc.vector.tensor_copy(h_r32[:], h_f[:])
                c_prev = c_new
                if DEBUG:
                    nc.sync.dma_start(out=dbg_h[t], in_=h_f[:])

                if t + 1 < n_steps:
                    bin_ = dram.tile([128, B], BF16, tag="bin", name=f"bin{t}")
                    nc.sync.dma_start(out=bin_[:], in_=h_r[:])
                    bout = dram.tile([NCORES * 128, B], BF16, tag="bout",
                                     name=f"bout{t}")
                    nc.gpsimd.collective_compute(
                        "AllGather", OP.bypass,
                        replica_groups=rg,
                        ins=[bin_.opt()], outs=[bout.opt()],
                    )
                    ht_tiles = []
                    for k in range(NCORES):
                        t_ = htp.tile([128, B], BF16, tag="ht",
                                      name=f"ht{t + 1}_{k}")
                        nc.sync.dma_start(
                            out=t_[:],
                            in_=bout[k * 128:(k + 1) * 128, :])
                        ht_tiles.append(t_)
                return h_r32

            def emit_logits(t, h_r):
                lg_ps = wkp.tile([128, 512], F32, tag="wk", name=f"lgp{t}")
                for half in range(2):
                    nc.tensor.matmul(
                        lg_ps[:, half * CP:(half + 1) * CP],
                        h_r[:, half * 128:(half + 1) * 128],
                        w_out_sb[:],
                        start=(half == 0), stop=(half == 1),
                        skip_group_check=True,
                    )
                for half in range(2):
                    lg_sb = lgs.tile([128, CP], F32, tag="lgs",
                                     name=f"lgs{t}_{half}")
                    nc.vector.tensor_copy(lg_sb[:], lg_ps[:, half * CP:(half + 1) * CP])
                    nc.gpsimd.dma_start(out=out[t, half], in_=lg_sb[:])

            # two alternating sets of 6 pi banks? No -- single set, but the
            # pool gives fresh tiles per group; allocate per group below.
            pi_banks_cur = {}

            def alloc_banks(g):
                pi_banks_cur[g % 2] = [
                    pip.tile([128, 2 * B], F32, tag=f"pi{gg}", name=f"pib{g}_{gg}")
                    for gg in range(6)]

            KH = KX // 2  # 16

            alloc_banks(0)
            emit_pi_half(0, 0, KX)
            pi_tiles = copy_pi_out(0)
            pending = []

            for g in range(n_grp):
                u, v = 2 * g, 2 * g + 1
                nxt = g + 1 < n_grp
                if nxt:
                    alloc_banks(g + 1)
                    emit_pi_half(g + 1, 0, KH)
                if pending:
                    emit_logits(*pending.pop())
                h_u = emit_step(u, pi_tiles, 0)
                if nxt:
                    emit_pi_half(g + 1, KH, KX)
                emit_logits(u, h_u)
                h_v = emit_step(v, pi_tiles, 1)
                pending.append((v, h_v))
                if nxt:
                    pi_tiles = copy_pi_out(g + 1)

            emit_logits(*pending.pop())

    nc.compile()
    return nc


def _prep_inputs(x, h0, c0, dropout_mask, W_in, b_in, W_state, b_state,
                 W_out, b_out):
    """Host-side shard + transpose + pad. Returns per-core input maps."""
    import ml_dtypes
    bf16 = ml_dtypes.bfloat16
    n_steps = x.shape[0]
    n_grp = n_steps // 2
    # x [T,B,DIN] -> [T, DINP, B] -> groups [T/2, KX, 128, 2B]
    xp = np.zeros((n_steps, DINP, B), dtype=np.float32)
    xp[:, :DIN, :] = x.transpose(0, 2, 1)
    xp = xp.reshape(n_grp, 2, KX, 128, B).transpose(0, 2, 3, 1, 4)
    xT = np.ascontiguousarray(xp.reshape(n_grp, KX, 128, 2 * B))
    xT = xT.astype(bf16)

    h0T_f = np.ascontiguousarray(h0.T.reshape(NCORES, 128, B)).astype(np.float32)

    h0T_bf = h0T_f.astype(bf16)
    in_maps = []
    for c in range(NCORES):
        sl = slice(c * HC, (c + 1) * HC)
        wi = np.concatenate([W_in[g * H + c * HC:g * H + (c + 1) * HC]
                             for g in range(6)], axis=0)  # [768, DIN]
        wip = np.zeros((768, DINP), dtype=np.float32)
        wip[:, :DIN] = wi
        w_in_c = np.ascontiguousarray(wip.T.reshape(KX, 128, 768))
        w_in_c = w_in_c.astype(bf16)

        ws = np.concatenate([W_state[g * H + c * HC:g * H + (c + 1) * HC]
                             for g in range(5)], axis=0)  # [640, H]
        w_st_c = np.ascontiguousarray(ws.T.reshape(NCORES, 128, 640)).astype(bf16)

        wo = np.zeros((128, CP), dtype=np.float32)
        wo[:, :C] = W_out[:, sl].T
        bias_c = np.zeros((128, 8), dtype=np.float32)
        for g in range(6):
            bias_c[:, g] = b_in[g * H + c * HC:g * H + (c + 1) * HC]
            if g < 5:
                bias_c[:, g] += b_state[g * H + c * HC:g * H + (c + 1) * HC]

        in_maps.append({
            "xT": xT,
            "w_in": w_in_c,
            "w_st": w_st_c,
            "w_out": wo,
            "bias": bias_c,
            "maskT": np.ascontiguousarray(dropout_mask.T[sl]).astype(np.float32),
            "c0T": np.ascontiguousarray(c0.T[sl]).astype(np.float32),
            "h0T": h0T_bf,
        })
    return in_maps


def kernel(x, h0, c0, dropout_mask, W_in, b_in, W_state, b_state,
           W_out, b_out, _trace=False):
    n_steps = x.shape[0]
    if n_steps not in _CACHE:
        _CACHE[n_steps] = build_nc(n_steps)
    nc = _CACHE[n_steps]
    in_maps = _prep_inputs(np.asarray(x, dtype=np.float32),
                           np.asarray(h0, dtype=np.float32),
                           np.asarray(c0, dtype=np.float32),
                           np.asarray(dropout_mask, dtype=np.float32),
                           np.asarray(W_in, dtype=np.float32),
                           np.asarray(b_in, dtype=np.float32),
                           np.asarray(W_state, dtype=np.float32),
                           np.asarray(b_state, dtype=np.float32),
                           np.asarray(W_out, dtype=np.float32),
                           np.asarray(b_out, dtype=np.float32))
    res = run_bass_kernel_spmd(nc, in_maps, list(range(NCORES)), trace=_trace)
    acc = np.zeros((n_steps, 2, 128, CP), dtype=np.float64)
    for r in res.results:
        acc += r["out"]
    logits = acc.reshape(n_steps, B, CP)[:, :, :C].astype(np.float32)
    logits += np.asarray(b_out, dtype=np.float32)
    kernel.last_result = res
    return logits
